# revision 2
# baseline (speedup 1.0000x reference)
"""Trainium2 Bass kernel for nn_Mesh2_14267881357853 (gnn_message_passing).

Computation (reference):
    out3 = concat(out1, out2) @ W_comb.T + b_comb              [N, 512]
    agg  = (out2 + sum_j out2[neighbour[:, j]]) * 0.25         [N, 256]
    out4 = agg @ W_agg.T + b_agg                               [N, 512]

Strategy: data-parallel over nodes, 8 cores x 25088 rows (25000 real + pad),
weights replicated, full out2 (bf16) replicated per core for the neighbour
gathers (SWDGE indirect DMA, one index per partition per instruction; the
three gathers accumulate in bf16 via the SDMA compute path so no engine
touches them).

The schedule is balanced against the CoreSim cost model, where every DMA
costs max(bytes_per_partition*0.3855ns, 500ns) on its ISSUING engine and the
Pool engine owns the three 500ns indirect gathers per tile (the binding
resource, 294us):
  Pool 1500/tile  3 gathers (hard floor: 1 index/partition/instruction)
  PE   1387/tile  4+2 matmul K-chunks (512 cols each) + 2 bf16 transposes
  DVE  1243/tile  aggt=pt+a2 (bf16 2x mode), o3 evict+f32 bias, o4 bf16 bias
  ACT  1007/tile  o4 PSUM->bf16 eviction + o4 store issue
  SP    790/tile  x1/a2 loads + o3 store issue
Gather sums, transposes, aggt and all bias adds run in bf16 (rel err ~0.006,
limit 2e-2). Outputs stored bf16, upcast to f32 on the host.
Cost-model time for the 8-core NEFF: ~306.5us (Pool 95.9% occupied).
"""

import numpy as np
import ml_dtypes
from contextlib import ExitStack

import concourse.bass as bass
import concourse.tile as tile
from concourse import bacc, mybir
from concourse.bass_utils import run_bass_kernel_spmd
from concourse.masks import make_identity

BF16 = ml_dtypes.bfloat16
P = 128
NCORES = 8
N_FULL = 200000
RPC = N_FULL // NCORES          # 25000 real rows per core
TB = 7                          # node-tiles per batch
NB = 28                         # batches  -> 196 tiles = 25088 padded rows
NP_PAD = NB * TB * P
D_IN = 256
D_OUT = 512


def build_program(nb=NB, tb=TB, n_full=N_FULL, n_cores=NCORES, timing=False,
                  lb=3, wb=10, ob=4, pstb=2, psmb=3, store_group=2):
    dt = mybir.dt
    npad = nb * tb * P
    nc = bacc.Bacc(
        "TRN2",
        target_bir_lowering=False,
        debug=False,
        enable_asserts=True,
        num_devices=n_cores,
    )
    x1d = nc.dram_tensor("x1t", [nb, P, tb * 2, P], dt.bfloat16, kind="ExternalInput").ap()
    a2d = nc.dram_tensor("a2t", [nb, P, tb * 2, P], dt.bfloat16, kind="ExternalInput").ap()
    idxd = nc.dram_tensor("idx", [P, nb * tb * 3], dt.int32, kind="ExternalInput").ap()
    o2fd = nc.dram_tensor("o2f", [n_full, D_IN], dt.bfloat16, kind="ExternalInput").ap()
    wctd = nc.dram_tensor("wct", [4, P, D_OUT], dt.bfloat16, kind="ExternalInput").ap()
    wagd = nc.dram_tensor("wagt", [2, P, D_OUT], dt.bfloat16, kind="ExternalInput").ap()
    bcd = nc.dram_tensor("bc", [P, D_OUT], dt.bfloat16, kind="ExternalInput").ap()
    babd = nc.dram_tensor("bab", [P, D_OUT], dt.bfloat16, kind="ExternalInput").ap()
    okind = "Internal" if timing else "ExternalOutput"
    o3d = nc.dram_tensor("o3", [npad, D_OUT], dt.bfloat16, kind=okind).ap()
    o4d = nc.dram_tensor("o4", [npad, D_OUT], dt.bfloat16, kind=okind).ap()
    chkd = (nc.dram_tensor("chk", [P, D_OUT], dt.float32, kind="ExternalOutput").ap()
            if timing else None)

    sg = store_group
    with tile.TileContext(nc) as tc, ExitStack() as ctx:
        const = ctx.enter_context(tc.tile_pool(name="const", bufs=1))
        loadp = ctx.enter_context(tc.tile_pool(name="loads", bufs=lb))
        work = ctx.enter_context(tc.tile_pool(name="work", bufs=wb))
        outp = ctx.enter_context(tc.tile_pool(name="outs", bufs=ob))
        pst = ctx.enter_context(tc.tile_pool(name="pst", bufs=pstb, space="PSUM"))
        psm = ctx.enter_context(tc.tile_pool(name="psm", bufs=psmb, space="PSUM"))

        idx_sb = const.tile([P, nb * tb * 3], dt.int32)
        nc.sync.dma_start(idx_sb[:], idxd[:])

        ident = const.tile([P, P], dt.float32)
        make_identity(nc, ident[:])
        identb = const.tile([P, P], dt.bfloat16)
        nc.vector.tensor_copy(identb[:], ident[:])

        wct_sb = const.tile([P, 4, D_OUT], dt.bfloat16)
        for c in range(4):
            nc.sync.dma_start(wct_sb[:, c, :], wctd[c])
        wag_sb = const.tile([P, 2, D_OUT], dt.bfloat16)
        for c in range(2):
            nc.sync.dma_start(wag_sb[:, c, :], wagd[c])
        bc_sb = const.tile([P, D_OUT], dt.bfloat16)
        nc.sync.dma_start(bc_sb[:], bcd[:])
        bab_sb = const.tile([P, D_OUT], dt.bfloat16)
        nc.sync.dma_start(bab_sb[:], babd[:])

        o3v = o3d.rearrange("(m p) d -> p m d", p=P)
        o4v = o4d.rearrange("(m p) d -> p m d", p=P)

        nt = nb * tb
        for m in range(nt):
            b, t = divmod(m, tb)
            if t == 0:
                x1 = loadp.tile([P, tb * 2, P], dt.bfloat16, tag="x1")
                nc.sync.dma_start(x1[:], x1d[b])
                a2 = loadp.tile([P, tb * 2, P], dt.bfloat16, tag="a2")
                nc.sync.dma_start(a2[:], a2d[b])
            # 3 neighbour rows, SDMA-side bf16 accumulate (one idx/partition)
            ki = m * 3
            gsum = work.tile([P, D_IN], dt.bfloat16, tag="gsum")
            for j in range(3):
                nc.gpsimd.indirect_dma_start(
                    out=gsum[:], out_offset=None, in_=o2fd[:],
                    in_offset=bass.IndirectOffsetOnAxis(
                        ap=idx_sb[:, ki + j:ki + j + 1], axis=0),
                    compute_op=(mybir.AluOpType.bypass if j == 0
                                else mybir.AluOpType.add),
                )
            # PE: bf16 transpose of the neighbour-sum to feature-major
            pt = pst.tile([P, 2, P], dt.bfloat16, tag="pt")
            for c in range(2):
                nc.tensor.matmul(
                    out=pt[:, c, :], lhsT=gsum[:, c * P:(c + 1) * P],
                    rhs=identb[:], is_transpose=True,
                    start=True, stop=True,
                )
            # aggt = pt + a2 (self rows); all-bf16 -> DVE 2x mode
            aggt = work.tile([P, 2, P], dt.bfloat16, tag="aggt")
            nc.vector.tensor_tensor(
                out=aggt[:], in0=pt[:], in1=a2[:, t * 2:t * 2 + 2, :],
                op=mybir.AluOpType.add,
            )
            # out3 = concat(out1,out2) @ W_comb.T  (4 K-chunks of 128)
            p3 = psm.tile([P, D_OUT], dt.float32, tag="p3")
            for c in range(4):
                lhsT = x1[:, t * 2 + c, :] if c < 2 else a2[:, t * 2 + c - 2, :]
                nc.tensor.matmul(
                    out=p3[:], lhsT=lhsT, rhs=wct_sb[:, c, :],
                    start=(c == 0), stop=(c == 3),
                )
            # out4 = aggt @ (0.25 W_agg).T  (2 K-chunks)
            p4 = psm.tile([P, D_OUT], dt.float32, tag="p4")
            for c in range(2):
                nc.tensor.matmul(
                    out=p4[:], lhsT=aggt[:, c, :], rhs=wag_sb[:, c, :],
                    start=(c == 0), stop=(c == 1),
                )
            sl = m % sg
            if sl == 0:
                o3sb = outp.tile([P, sg, D_OUT], dt.bfloat16, tag="o3sb")
                o4sb = outp.tile([P, sg, D_OUT], dt.bfloat16, tag="o4sb")
            # o3 eviction with fused bias on DVE
            nc.vector.tensor_tensor(
                out=o3sb[:, sl, :], in0=p3[:], in1=bc_sb[:],
                op=mybir.AluOpType.add)
            # o4 eviction: ACT PSUM->bf16 copy, then DVE bf16 bias add (2x)
            o4nb = work.tile([P, D_OUT], dt.bfloat16, tag="o4nb")
            nc.scalar.copy(out=o4nb[:], in_=p4[:])
            nc.vector.tensor_tensor(
                out=o4sb[:, sl, :], in0=o4nb[:], in1=bab_sb[:],
                op=mybir.AluOpType.add)
            if sl == sg - 1 or m == nt - 1:
                m0 = m - sl
                nc.sync.dma_start(o3v[:, m0:m + 1, :], o3sb[:, :sl + 1, :])
                nc.scalar.dma_start(o4v[:, m0:m + 1, :], o4sb[:, :sl + 1, :])

        if timing:
            chk = outp.tile([P, D_OUT], dt.float32, tag="chk")
            nc.vector.tensor_tensor(out=chk[:], in0=o3sb[:, 0, :],
                                    in1=o4sb[:, 0, :], op=mybir.AluOpType.add)
            nc.sync.dma_start(chkd[:], chk[:])

    nc.compile()
    return nc


def _pack_T(rows, nb, tb):
    """[rows, 256] f32 -> [nb, P, tb*2, P] bf16 feature-major tiles."""
    npad = nb * tb * P
    pad = np.zeros((npad, D_IN), BF16)
    pad[: rows.shape[0]] = rows.astype(BF16)
    r = pad.reshape(nb, tb, P, 2, P)               # [b, t, node, c, feat]
    return np.ascontiguousarray(r.transpose(0, 4, 1, 3, 2)).reshape(nb, P, tb * 2, P)


def _pack_idx(nbr, nb, tb):
    """[rows, 3] int32 -> [P, nb*tb*3] partition-major index layout."""
    npad = nb * tb * P
    pad = np.zeros((npad, 3), np.int32)
    pad[: nbr.shape[0]] = nbr
    r = pad.reshape(nb, tb, P, 3)                  # [b, t, node, j]
    return np.ascontiguousarray(r.transpose(2, 0, 1, 3).reshape(P, nb * tb * 3))


def prep_in_maps(out1, out2, neighbour, W_comb, b_comb, W_agg, b_agg,
                 nb=NB, tb=TB, n_cores=NCORES):
    out1 = np.asarray(out1, dtype=np.float32)
    out2 = np.asarray(out2, dtype=np.float32)
    nbr32 = np.asarray(neighbour).astype(np.int32)
    o2f = np.ascontiguousarray(out2.astype(BF16))
    wct = np.ascontiguousarray(np.asarray(W_comb, dtype=np.float32).T.astype(BF16)).reshape(4, P, D_OUT)
    wag = np.ascontiguousarray((0.25 * np.asarray(W_agg, dtype=np.float32)).T.astype(BF16)).reshape(2, P, D_OUT)
    bc = np.ascontiguousarray(
        np.tile(np.asarray(b_comb, dtype=np.float32).astype(BF16)[None, :], (P, 1)))
    bab = np.ascontiguousarray(
        np.tile(np.asarray(b_agg, dtype=np.float32).astype(BF16)[None, :], (P, 1)))
    rpc = out1.shape[0] // n_cores
    in_maps = []
    for i in range(n_cores):
        sl = slice(i * rpc, (i + 1) * rpc)
        in_maps.append(dict(
            x1t=_pack_T(out1[sl], nb, tb),
            a2t=_pack_T(out2[sl], nb, tb),
            idx=_pack_idx(nbr32[sl], nb, tb),
            o2f=o2f, wct=wct, wagt=wag, bc=bc, bab=bab,
        ))
    return in_maps


_NC_CACHE = {}


def _get_program():
    key = (NB, TB, N_FULL)
    if key not in _NC_CACHE:
        _NC_CACHE[key] = build_program()
    return _NC_CACHE[key]


def kernel(out1, out2, neighbour, W_comb, b_comb, W_agg, b_agg, _trace=False, **kw):
    nc = _get_program()
    in_maps = prep_in_maps(out1, out2, neighbour, W_comb, b_comb, W_agg, b_agg)
    res = run_bass_kernel_spmd(nc, in_maps, list(range(NCORES)), trace=_trace, **kw)
    out3 = np.concatenate([res.results[i]["o3"][:RPC].astype(np.float32) for i in range(NCORES)], axis=0)
    out4 = np.concatenate([res.results[i]["o4"][:RPC].astype(np.float32) for i in range(NCORES)], axis=0)
    if _trace:
        return (out3, out4), res
    return (out3, out4)


# revision 4
# speedup vs baseline: 1.0056x; 1.0056x over previous
"""Trainium2 Bass kernel for nn_Mesh2_14267881357853 (gnn_message_passing).

Computation (reference):
    out3 = concat(out1, out2) @ W_comb.T + b_comb              [N, 512]
    agg  = (out2 + sum_j out2[neighbour[:, j]]) * 0.25         [N, 256]
    out4 = agg @ W_agg.T + b_agg                               [N, 512]

Strategy: data-parallel over nodes, 8 cores x 25088 rows (25000 real + pad),
weights replicated, full out2 (bf16) replicated per core for the neighbour
gathers (SWDGE indirect DMA, one index per partition per instruction; the
three gathers accumulate in bf16 via the SDMA compute path so no engine
touches them).

The schedule is balanced against the CoreSim cost model, where every DMA
costs max(bytes_per_partition*0.3855ns, 500ns) on its ISSUING engine and the
Pool engine owns the three 500ns indirect gathers per tile (the binding
resource, 294us):
  Pool 1500/tile  3 gathers (hard floor: 1 index/partition/instruction)
  PE   1387/tile  4+2 matmul K-chunks (512 cols each) + 2 bf16 transposes
  DVE  1243/tile  aggt=pt+a2 (bf16 2x mode), o3 evict+f32 bias, o4 bf16 bias
  ACT  1007/tile  o4 PSUM->bf16 eviction + o4 store issue
  SP    790/tile  x1/a2 loads + o3 store issue
Gather sums, transposes, aggt and all bias adds run in bf16 (rel err ~0.006,
limit 2e-2). Outputs stored bf16, upcast to f32 on the host.
Cost-model time for the 8-core NEFF: ~306.5us (Pool 95.9% occupied).
"""

import numpy as np
import ml_dtypes
from contextlib import ExitStack

import concourse.bass as bass
import concourse.tile as tile
from concourse import bacc, mybir
from concourse.bass_utils import run_bass_kernel_spmd
from concourse.masks import make_identity

BF16 = ml_dtypes.bfloat16
P = 128
NCORES = 8
N_FULL = 200000
RPC = N_FULL // NCORES          # 25000 real rows per core
TB = 7                          # node-tiles per batch
NB = 28                         # batches  -> 196 tiles = 25088 padded rows
NP_PAD = NB * TB * P
D_IN = 256
D_OUT = 512


def build_program(nb=NB, tb=TB, n_full=N_FULL, n_cores=NCORES, timing=False,
                  lb=3, wb=10, ob=4, pstb=2, psmb=3, store_group=2):
    dt = mybir.dt
    npad = nb * tb * P
    nc = bacc.Bacc(
        "TRN2",
        target_bir_lowering=False,
        debug=False,
        enable_asserts=True,
        num_devices=n_cores,
    )
    x1d = nc.dram_tensor("x1t", [nb, P, tb * 2, P], dt.bfloat16, kind="ExternalInput").ap()
    a2d = nc.dram_tensor("a2t", [nb, P, tb * 2, P], dt.bfloat16, kind="ExternalInput").ap()
    idxd = nc.dram_tensor("idx", [P, nb * tb * 3], dt.int32, kind="ExternalInput").ap()
    o2fd = nc.dram_tensor("o2f", [n_full, D_IN], dt.bfloat16, kind="ExternalInput").ap()
    wctd = nc.dram_tensor("wct", [4, P, D_OUT], dt.bfloat16, kind="ExternalInput").ap()
    wagd = nc.dram_tensor("wagt", [2, P, D_OUT], dt.bfloat16, kind="ExternalInput").ap()
    bcd = nc.dram_tensor("bc", [P, D_OUT], dt.bfloat16, kind="ExternalInput").ap()
    babd = nc.dram_tensor("bab", [P, D_OUT], dt.bfloat16, kind="ExternalInput").ap()
    okind = "Internal" if timing else "ExternalOutput"
    o3d = nc.dram_tensor("o3", [npad, D_OUT], dt.bfloat16, kind=okind).ap()
    o4d = nc.dram_tensor("o4", [npad, D_OUT], dt.bfloat16, kind=okind).ap()
    chkd = (nc.dram_tensor("chk", [P, D_OUT], dt.float32, kind="ExternalOutput").ap()
            if timing else None)

    sg = store_group
    with tile.TileContext(nc) as tc, ExitStack() as ctx:
        const = ctx.enter_context(tc.tile_pool(name="const", bufs=1))
        loadp = ctx.enter_context(tc.tile_pool(name="loads", bufs=lb))
        work = ctx.enter_context(tc.tile_pool(name="work", bufs=wb))
        outp = ctx.enter_context(tc.tile_pool(name="outs", bufs=ob))
        pst = ctx.enter_context(tc.tile_pool(name="pst", bufs=pstb, space="PSUM"))
        psm = ctx.enter_context(tc.tile_pool(name="psm", bufs=psmb, space="PSUM"))

        # split the index load so the first tiles' gathers start early
        idx_sb = const.tile([P, nb * tb * 3], dt.int32)
        ic = 3 * tb
        nc.sync.dma_start(idx_sb[:, :ic], idxd[:, :ic])
        nc.sync.dma_start(idx_sb[:, ic:], idxd[:, ic:])

        ident = const.tile([P, P], dt.float32)
        make_identity(nc, ident[:])
        identb = const.tile([P, P], dt.bfloat16)
        nc.vector.tensor_copy(identb[:], ident[:])

        wct_sb = const.tile([P, 4, D_OUT], dt.bfloat16)
        for c in range(4):
            nc.sync.dma_start(wct_sb[:, c, :], wctd[c])
        wag_sb = const.tile([P, 2, D_OUT], dt.bfloat16)
        for c in range(2):
            nc.sync.dma_start(wag_sb[:, c, :], wagd[c])
        bc_sb = const.tile([P, D_OUT], dt.bfloat16)
        nc.sync.dma_start(bc_sb[:], bcd[:])
        bab_sb = const.tile([P, D_OUT], dt.bfloat16)
        nc.sync.dma_start(bab_sb[:], babd[:])

        o3v = o3d.rearrange("(m p) d -> p m d", p=P)
        o4v = o4d.rearrange("(m p) d -> p m d", p=P)

        nt = nb * tb
        for m in range(nt):
            b, t = divmod(m, tb)
            if t == 0:
                x1 = loadp.tile([P, tb * 2, P], dt.bfloat16, tag="x1")
                nc.sync.dma_start(x1[:], x1d[b])
                a2 = loadp.tile([P, tb * 2, P], dt.bfloat16, tag="a2")
                nc.sync.dma_start(a2[:], a2d[b])
            # 3 neighbour rows, SDMA-side bf16 accumulate (one idx/partition)
            ki = m * 3
            gsum = work.tile([P, D_IN], dt.bfloat16, tag="gsum")
            for j in range(3):
                nc.gpsimd.indirect_dma_start(
                    out=gsum[:], out_offset=None, in_=o2fd[:],
                    in_offset=bass.IndirectOffsetOnAxis(
                        ap=idx_sb[:, ki + j:ki + j + 1], axis=0),
                    compute_op=(mybir.AluOpType.bypass if j == 0
                                else mybir.AluOpType.add),
                )
            # PE: bf16 transpose of the neighbour-sum to feature-major
            pt = pst.tile([P, 2, P], dt.bfloat16, tag="pt")
            for c in range(2):
                nc.tensor.matmul(
                    out=pt[:, c, :], lhsT=gsum[:, c * P:(c + 1) * P],
                    rhs=identb[:], is_transpose=True,
                    start=True, stop=True,
                )
            # aggt = pt + a2 (self rows); all-bf16 -> DVE 2x mode
            aggt = work.tile([P, 2, P], dt.bfloat16, tag="aggt")
            nc.vector.tensor_tensor(
                out=aggt[:], in0=pt[:], in1=a2[:, t * 2:t * 2 + 2, :],
                op=mybir.AluOpType.add,
            )
            # out3 = concat(out1,out2) @ W_comb.T  (4 K-chunks of 128)
            p3 = psm.tile([P, D_OUT], dt.float32, tag="p3")
            for c in range(4):
                lhsT = x1[:, t * 2 + c, :] if c < 2 else a2[:, t * 2 + c - 2, :]
                nc.tensor.matmul(
                    out=p3[:], lhsT=lhsT, rhs=wct_sb[:, c, :],
                    start=(c == 0), stop=(c == 3),
                )
            # out4 = aggt @ (0.25 W_agg).T  (2 K-chunks)
            p4 = psm.tile([P, D_OUT], dt.float32, tag="p4")
            for c in range(2):
                nc.tensor.matmul(
                    out=p4[:], lhsT=aggt[:, c, :], rhs=wag_sb[:, c, :],
                    start=(c == 0), stop=(c == 1),
                )
            sl = m % sg
            if sl == 0:
                o3sb = outp.tile([P, sg, D_OUT], dt.bfloat16, tag="o3sb")
                o4sb = outp.tile([P, sg, D_OUT], dt.bfloat16, tag="o4sb")
            # o3 eviction with fused bias on DVE
            nc.vector.tensor_tensor(
                out=o3sb[:, sl, :], in0=p3[:], in1=bc_sb[:],
                op=mybir.AluOpType.add)
            if m >= nt - 2:
                # tail tiles: single fused DVE eviction (shorter final chain)
                nc.vector.tensor_tensor(
                    out=o4sb[:, sl, :], in0=p4[:], in1=bab_sb[:],
                    op=mybir.AluOpType.add)
            else:
                # o4 eviction: ACT PSUM->bf16 copy, DVE bf16 bias add (2x)
                o4nb = work.tile([P, D_OUT], dt.bfloat16, tag="o4nb")
                nc.scalar.copy(out=o4nb[:], in_=p4[:])
                nc.vector.tensor_tensor(
                    out=o4sb[:, sl, :], in0=o4nb[:], in1=bab_sb[:],
                    op=mybir.AluOpType.add)
            if sl == sg - 1 or m == nt - 1:
                m0 = m - sl
                nc.sync.dma_start(o3v[:, m0:m + 1, :], o3sb[:, :sl + 1, :])
                nc.scalar.dma_start(o4v[:, m0:m + 1, :], o4sb[:, :sl + 1, :])

        if timing:
            chk = outp.tile([P, D_OUT], dt.float32, tag="chk")
            nc.vector.tensor_tensor(out=chk[:], in0=o3sb[:, 0, :],
                                    in1=o4sb[:, 0, :], op=mybir.AluOpType.add)
            nc.sync.dma_start(chkd[:], chk[:])

    nc.compile()
    return nc


def _pack_T(rows, nb, tb):
    """[rows, 256] f32 -> [nb, P, tb*2, P] bf16 feature-major tiles."""
    npad = nb * tb * P
    pad = np.zeros((npad, D_IN), BF16)
    pad[: rows.shape[0]] = rows.astype(BF16)
    r = pad.reshape(nb, tb, P, 2, P)               # [b, t, node, c, feat]
    return np.ascontiguousarray(r.transpose(0, 4, 1, 3, 2)).reshape(nb, P, tb * 2, P)


def _pack_idx(nbr, nb, tb):
    """[rows, 3] int32 -> [P, nb*tb*3] partition-major index layout."""
    npad = nb * tb * P
    pad = np.zeros((npad, 3), np.int32)
    pad[: nbr.shape[0]] = nbr
    r = pad.reshape(nb, tb, P, 3)                  # [b, t, node, j]
    return np.ascontiguousarray(r.transpose(2, 0, 1, 3).reshape(P, nb * tb * 3))


def prep_in_maps(out1, out2, neighbour, W_comb, b_comb, W_agg, b_agg,
                 nb=NB, tb=TB, n_cores=NCORES):
    out1 = np.asarray(out1, dtype=np.float32)
    out2 = np.asarray(out2, dtype=np.float32)
    nbr32 = np.asarray(neighbour).astype(np.int32)
    o2f = np.ascontiguousarray(out2.astype(BF16))
    wct = np.ascontiguousarray(np.asarray(W_comb, dtype=np.float32).T.astype(BF16)).reshape(4, P, D_OUT)
    wag = np.ascontiguousarray((0.25 * np.asarray(W_agg, dtype=np.float32)).T.astype(BF16)).reshape(2, P, D_OUT)
    bc = np.ascontiguousarray(
        np.tile(np.asarray(b_comb, dtype=np.float32).astype(BF16)[None, :], (P, 1)))
    bab = np.ascontiguousarray(
        np.tile(np.asarray(b_agg, dtype=np.float32).astype(BF16)[None, :], (P, 1)))
    rpc = out1.shape[0] // n_cores
    in_maps = []
    for i in range(n_cores):
        sl = slice(i * rpc, (i + 1) * rpc)
        in_maps.append(dict(
            x1t=_pack_T(out1[sl], nb, tb),
            a2t=_pack_T(out2[sl], nb, tb),
            idx=_pack_idx(nbr32[sl], nb, tb),
            o2f=o2f, wct=wct, wagt=wag, bc=bc, bab=bab,
        ))
    return in_maps


_NC_CACHE = {}


def _get_program():
    key = (NB, TB, N_FULL)
    if key not in _NC_CACHE:
        _NC_CACHE[key] = build_program()
    return _NC_CACHE[key]


def kernel(out1, out2, neighbour, W_comb, b_comb, W_agg, b_agg, _trace=False, **kw):
    nc = _get_program()
    in_maps = prep_in_maps(out1, out2, neighbour, W_comb, b_comb, W_agg, b_agg)
    res = run_bass_kernel_spmd(nc, in_maps, list(range(NCORES)), trace=_trace, **kw)
    out3 = np.concatenate([res.results[i]["o3"][:RPC].astype(np.float32) for i in range(NCORES)], axis=0)
    out4 = np.concatenate([res.results[i]["o4"][:RPC].astype(np.float32) for i in range(NCORES)], axis=0)
    if _trace:
        return (out3, out4), res
    return (out3, out4)


# revision 8
# speedup vs baseline: 1.0074x; 1.0018x over previous
"""Trainium2 Bass kernel for nn_Mesh2_14267881357853 (gnn_message_passing).

Computation (reference):
    out3 = concat(out1, out2) @ W_comb.T + b_comb              [N, 512]
    agg  = (out2 + sum_j out2[neighbour[:, j]]) * 0.25         [N, 256]
    out4 = agg @ W_agg.T + b_agg                               [N, 512]

Strategy: data-parallel over nodes, 8 cores x 25088 rows (25000 real + pad),
weights replicated, full out2 (bf16) replicated per core for the neighbour
gathers (SWDGE indirect DMA, one index per partition per instruction; the
three gathers accumulate in bf16 via the SDMA compute path so no engine
touches them).

The schedule is balanced against the CoreSim cost model, where every DMA
costs max(bytes_per_partition*0.3855ns, 500ns) on its ISSUING engine and the
Pool engine owns the three 500ns indirect gathers per tile (the binding
resource, 294us):
  Pool 1500/tile  3 gathers (hard floor: 1 index/partition/instruction)
  PE   1387/tile  4+2 matmul K-chunks (512 cols each) + 2 bf16 transposes
  DVE  1243/tile  aggt=pt+a2 (bf16 2x mode), o3 evict+f32 bias, o4 bf16 bias
  ACT  1007/tile  o4 PSUM->bf16 eviction + o4 store issue
  SP    790/tile  x1/a2 loads + o3 store issue
Gather sums, transposes, aggt and all bias adds run in bf16 (rel err ~0.006,
limit 2e-2). Outputs stored bf16, upcast to f32 on the host. The index load
is split so the first gathers start early, and the last two tiles use a
single fused DVE eviction to shorten the final dependency chain.
Cost-model time for the 8-core NEFF: ~304.3us (Pool ~96.6% occupied;
294us of it is the hard 3x500ns/tile gather floor).
"""

import numpy as np
import ml_dtypes
from contextlib import ExitStack

import concourse.bass as bass
import concourse.tile as tile
from concourse import bacc, mybir
from concourse.bass_utils import run_bass_kernel_spmd
from concourse.masks import make_identity

BF16 = ml_dtypes.bfloat16
P = 128
NCORES = 8
N_FULL = 200000
RPC = N_FULL // NCORES          # 25000 real rows per core
TB = 7                          # node-tiles per batch
NB = 28                         # batches  -> 196 tiles = 25088 padded rows
NP_PAD = NB * TB * P
D_IN = 256
D_OUT = 512


def build_program(nb=NB, tb=TB, n_full=N_FULL, n_cores=NCORES, timing=False,
                  lb=3, wb=10, ob=4, pstb=2, psmb=2, store_group=2):
    dt = mybir.dt
    npad = nb * tb * P
    nc = bacc.Bacc(
        "TRN2",
        target_bir_lowering=False,
        debug=False,
        enable_asserts=True,
        num_devices=n_cores,
    )
    x1d = nc.dram_tensor("x1t", [nb, P, tb * 2, P], dt.bfloat16, kind="ExternalInput").ap()
    a2d = nc.dram_tensor("a2t", [nb, P, tb * 2, P], dt.bfloat16, kind="ExternalInput").ap()
    idxd = nc.dram_tensor("idx", [P, nb * tb * 3], dt.int32, kind="ExternalInput").ap()
    o2fd = nc.dram_tensor("o2f", [n_full, D_IN], dt.bfloat16, kind="ExternalInput").ap()
    wctd = nc.dram_tensor("wct", [4, P, D_OUT], dt.bfloat16, kind="ExternalInput").ap()
    wagd = nc.dram_tensor("wagt", [2, P, D_OUT], dt.bfloat16, kind="ExternalInput").ap()
    bcd = nc.dram_tensor("bc", [P, D_OUT], dt.bfloat16, kind="ExternalInput").ap()
    babd = nc.dram_tensor("bab", [P, D_OUT], dt.bfloat16, kind="ExternalInput").ap()
    okind = "Internal" if timing else "ExternalOutput"
    o3d = nc.dram_tensor("o3", [npad, D_OUT], dt.bfloat16, kind=okind).ap()
    o4d = nc.dram_tensor("o4", [npad, D_OUT], dt.bfloat16, kind=okind).ap()
    chkd = (nc.dram_tensor("chk", [P, D_OUT], dt.float32, kind="ExternalOutput").ap()
            if timing else None)

    sg = store_group
    with tile.TileContext(nc) as tc, ExitStack() as ctx:
        const = ctx.enter_context(tc.tile_pool(name="const", bufs=1))
        loadp = ctx.enter_context(tc.tile_pool(name="loads", bufs=lb))
        work = ctx.enter_context(tc.tile_pool(name="work", bufs=wb))
        outp = ctx.enter_context(tc.tile_pool(name="outs", bufs=ob))
        pst = ctx.enter_context(tc.tile_pool(name="pst", bufs=pstb, space="PSUM"))
        psm = ctx.enter_context(tc.tile_pool(name="psm", bufs=psmb, space="PSUM"))

        # split the index load so the first tiles' gathers start early
        idx_sb = const.tile([P, nb * tb * 3], dt.int32)
        ic = 3 * tb
        nc.sync.dma_start(idx_sb[:, :ic], idxd[:, :ic])
        nc.sync.dma_start(idx_sb[:, ic:], idxd[:, ic:])

        ident = const.tile([P, P], dt.float32)
        make_identity(nc, ident[:])
        identb = const.tile([P, P], dt.bfloat16)
        nc.vector.tensor_copy(identb[:], ident[:])

        wct_sb = const.tile([P, 4, D_OUT], dt.bfloat16)
        for c in range(4):
            nc.sync.dma_start(wct_sb[:, c, :], wctd[c])
        wag_sb = const.tile([P, 2, D_OUT], dt.bfloat16)
        for c in range(2):
            nc.sync.dma_start(wag_sb[:, c, :], wagd[c])
        bc_sb = const.tile([P, D_OUT], dt.bfloat16)
        nc.sync.dma_start(bc_sb[:], bcd[:])
        bab_sb = const.tile([P, D_OUT], dt.bfloat16)
        nc.sync.dma_start(bab_sb[:], babd[:])

        o3v = o3d.rearrange("(m p) d -> p m d", p=P)
        o4v = o4d.rearrange("(m p) d -> p m d", p=P)

        nt = nb * tb
        for m in range(nt):
            b, t = divmod(m, tb)
            if t == 0:
                x1 = loadp.tile([P, tb * 2, P], dt.bfloat16, tag="x1")
                nc.sync.dma_start(x1[:], x1d[b])
                a2 = loadp.tile([P, tb * 2, P], dt.bfloat16, tag="a2")
                nc.sync.dma_start(a2[:], a2d[b])
            # 3 neighbour rows, SDMA-side bf16 accumulate (one idx/partition)
            ki = m * 3
            gsum = work.tile([P, D_IN], dt.bfloat16, tag="gsum")
            for j in range(3):
                nc.gpsimd.indirect_dma_start(
                    out=gsum[:], out_offset=None, in_=o2fd[:],
                    in_offset=bass.IndirectOffsetOnAxis(
                        ap=idx_sb[:, ki + j:ki + j + 1], axis=0),
                    compute_op=(mybir.AluOpType.bypass if j == 0
                                else mybir.AluOpType.add),
                )
            # PE: bf16 transpose of the neighbour-sum to feature-major
            pt = pst.tile([P, 2, P], dt.bfloat16, tag="pt")
            for c in range(2):
                nc.tensor.matmul(
                    out=pt[:, c, :], lhsT=gsum[:, c * P:(c + 1) * P],
                    rhs=identb[:], is_transpose=True,
                    start=True, stop=True,
                )
            # aggt = pt + a2 (self rows); all-bf16 -> DVE 2x mode
            aggt = work.tile([P, 2, P], dt.bfloat16, tag="aggt")
            nc.vector.tensor_tensor(
                out=aggt[:], in0=pt[:], in1=a2[:, t * 2:t * 2 + 2, :],
                op=mybir.AluOpType.add,
            )
            # out3 = concat(out1,out2) @ W_comb.T  (4 K-chunks of 128)
            p3 = psm.tile([P, D_OUT], dt.float32, tag="p3")
            for c in range(4):
                lhsT = x1[:, t * 2 + c, :] if c < 2 else a2[:, t * 2 + c - 2, :]
                nc.tensor.matmul(
                    out=p3[:], lhsT=lhsT, rhs=wct_sb[:, c, :],
                    start=(c == 0), stop=(c == 3),
                )
            # out4 = aggt @ (0.25 W_agg).T  (2 K-chunks)
            p4 = psm.tile([P, D_OUT], dt.float32, tag="p4")
            for c in range(2):
                nc.tensor.matmul(
                    out=p4[:], lhsT=aggt[:, c, :], rhs=wag_sb[:, c, :],
                    start=(c == 0), stop=(c == 1),
                )
            sl = 0 if m >= nt - 2 else m % sg
            if sl == 0:
                o3sb = outp.tile([P, sg, D_OUT], dt.bfloat16, tag="o3sb")
                o4sb = outp.tile([P, sg, D_OUT], dt.bfloat16, tag="o4sb")
            # o3 eviction with fused bias on DVE
            nc.vector.tensor_tensor(
                out=o3sb[:, sl, :], in0=p3[:], in1=bc_sb[:],
                op=mybir.AluOpType.add)
            if m >= nt - 2:
                # tail tiles: single fused DVE eviction (shorter final chain)
                nc.vector.tensor_tensor(
                    out=o4sb[:, sl, :], in0=p4[:], in1=bab_sb[:],
                    op=mybir.AluOpType.add)
            else:
                # o4 eviction: ACT PSUM->bf16 copy, DVE bf16 bias add (2x)
                o4nb = work.tile([P, D_OUT], dt.bfloat16, tag="o4nb")
                nc.scalar.copy(out=o4nb[:], in_=p4[:])
                nc.vector.tensor_tensor(
                    out=o4sb[:, sl, :], in0=o4nb[:], in1=bab_sb[:],
                    op=mybir.AluOpType.add)
            if sl == sg - 1 or m >= nt - 2:
                m0 = m - sl
                nc.sync.dma_start(o3v[:, m0:m + 1, :], o3sb[:, :sl + 1, :])
                nc.scalar.dma_start(o4v[:, m0:m + 1, :], o4sb[:, :sl + 1, :])

        if timing:
            chk = outp.tile([P, D_OUT], dt.float32, tag="chk")
            nc.vector.tensor_tensor(out=chk[:], in0=o3sb[:, 0, :],
                                    in1=o4sb[:, 0, :], op=mybir.AluOpType.add)
            nc.sync.dma_start(chkd[:], chk[:])

    nc.compile()
    return nc


def _pack_T(rows, nb, tb):
    """[rows, 256] f32 -> [nb, P, tb*2, P] bf16 feature-major tiles."""
    npad = nb * tb * P
    pad = np.zeros((npad, D_IN), BF16)
    pad[: rows.shape[0]] = rows.astype(BF16)
    r = pad.reshape(nb, tb, P, 2, P)               # [b, t, node, c, feat]
    return np.ascontiguousarray(r.transpose(0, 4, 1, 3, 2)).reshape(nb, P, tb * 2, P)


def _pack_idx(nbr, nb, tb):
    """[rows, 3] int32 -> [P, nb*tb*3] partition-major index layout."""
    npad = nb * tb * P
    pad = np.zeros((npad, 3), np.int32)
    pad[: nbr.shape[0]] = nbr
    r = pad.reshape(nb, tb, P, 3)                  # [b, t, node, j]
    return np.ascontiguousarray(r.transpose(2, 0, 1, 3).reshape(P, nb * tb * 3))


def prep_in_maps(out1, out2, neighbour, W_comb, b_comb, W_agg, b_agg,
                 nb=NB, tb=TB, n_cores=NCORES):
    out1 = np.asarray(out1, dtype=np.float32)
    out2 = np.asarray(out2, dtype=np.float32)
    nbr32 = np.asarray(neighbour).astype(np.int32)
    o2f = np.ascontiguousarray(out2.astype(BF16))
    wct = np.ascontiguousarray(np.asarray(W_comb, dtype=np.float32).T.astype(BF16)).reshape(4, P, D_OUT)
    wag = np.ascontiguousarray((0.25 * np.asarray(W_agg, dtype=np.float32)).T.astype(BF16)).reshape(2, P, D_OUT)
    bc = np.ascontiguousarray(
        np.tile(np.asarray(b_comb, dtype=np.float32).astype(BF16)[None, :], (P, 1)))
    bab = np.ascontiguousarray(
        np.tile(np.asarray(b_agg, dtype=np.float32).astype(BF16)[None, :], (P, 1)))
    rpc = out1.shape[0] // n_cores
    in_maps = []
    for i in range(n_cores):
        sl = slice(i * rpc, (i + 1) * rpc)
        in_maps.append(dict(
            x1t=_pack_T(out1[sl], nb, tb),
            a2t=_pack_T(out2[sl], nb, tb),
            idx=_pack_idx(nbr32[sl], nb, tb),
            o2f=o2f, wct=wct, wagt=wag, bc=bc, bab=bab,
        ))
    return in_maps


_NC_CACHE = {}


def _get_program():
    key = (NB, TB, N_FULL)
    if key not in _NC_CACHE:
        _NC_CACHE[key] = build_program()
    return _NC_CACHE[key]


def kernel(out1, out2, neighbour, W_comb, b_comb, W_agg, b_agg, _trace=False, **kw):
    nc = _get_program()
    in_maps = prep_in_maps(out1, out2, neighbour, W_comb, b_comb, W_agg, b_agg)
    res = run_bass_kernel_spmd(nc, in_maps, list(range(NCORES)), trace=_trace, **kw)
    out3 = np.concatenate([res.results[i]["o3"][:RPC].astype(np.float32) for i in range(NCORES)], axis=0)
    out4 = np.concatenate([res.results[i]["o4"][:RPC].astype(np.float32) for i in range(NCORES)], axis=0)
    if _trace:
        return (out3, out4), res
    return (out3, out4)
